# revision 13
# baseline (speedup 1.0000x reference)
"""Trainium2 Bass kernel for the attention-LSTM decoder (nn_Attention).

Sharding: data-parallel over batch across 8 NeuronCores (32 batch rows each),
weights replicated.  All recurrent state and big tensors live in SBUF in bf16;
H_proj^T (the hoisted i2h projection) is computed on-device once, spilled to
device DRAM, and streamed back block-wise each step (SBUF is too small to hold
it alongside batch_H^T).

Per decode step (25 steps, fully unrolled):
  phT   = W_h2h @ h^T + b_h2h                       (PE, [512,32])
  th    = tanh(HpT + phT broadcast over t)          (DVE bcast-add + ACT tanh,
                                                     [512, 8192] in 8 blocks)
  e     = w_score . th  (contract over H)           (PE, M=1 rows -> [32,256])
  alpha = softmax(e) over t                         (DVE max, ACT exp+accum,
                                                     DVE recip)
  ctx   = alpha @ batch_H                           (PE via alpha^T transpose)
  gates = [ctx, char_e, h] @ Wg^T + b               (PE, K=1281 incl bias row)
  i,f,o via sigmoid(x)=0.5(1+tanh(x/2)), g via tanh (ACT, single table set)
  c,h updates                                       (DVE pointwise)
probs = hiddens @ W_gen^T + b_gen at the end (batched over all 25 steps).
"""

import numpy as np
import ml_dtypes

BL = 32        # local batch per core
T = 256
H = 512
INP = 512
EMB = 256
NCLS = 97
S = 25
NCORES = 8
BT = BL * T    # 8192
JBLK = 1024    # tanh free-dim block (4 batch rows worth)
NJ = BT // JBLK
BPB = JBLK // T  # batch rows per block

_cache = {}


def _build_nc():
    import concourse.bass as bass
    import concourse.tile as tile
    from concourse import bacc, mybir, masks
    from contextlib import ExitStack

    dt = mybir.dt
    F32, BF16 = dt.float32, dt.bfloat16
    AF = mybir.ActivationFunctionType
    OP = mybir.AluOpType
    AX = mybir.AxisListType

    nc = bacc.Bacc("TRN2", target_bir_lowering=False)

    # ---- I/O ----
    bh_i = nc.dram_tensor("bh_i", [INP, BT], BF16, kind="ExternalInput")
    bh_t = nc.dram_tensor("bh_t", [T, BL * INP], BF16, kind="ExternalInput")
    wi2h = nc.dram_tensor("wi2h", [INP, H], BF16, kind="ExternalInput")
    wh2h = nc.dram_tensor("wh2h", [H, H], BF16, kind="ExternalInput")
    bh2h = nc.dram_tensor("bh2h", [128, 4], F32, kind="ExternalInput")
    wsc = nc.dram_tensor("wsc", [128, 4 * 1024], BF16, kind="ExternalInput")
    wg = nc.dram_tensor("wg", [INP + EMB + H, 4 * H], BF16, kind="ExternalInput")
    bgrow = nc.dram_tensor("bgrow", [1, 4 * H], BF16, kind="ExternalInput")
    ce = nc.dram_tensor("ce", [EMB, S * BL], BF16, kind="ExternalInput")
    wgen = nc.dram_tensor("wgen", [H, NCLS], BF16, kind="ExternalInput")
    bgen = nc.dram_tensor("bgen", [NCLS, 1], F32, kind="ExternalInput")

    attn_o = nc.dram_tensor("attn_o", [S, BL, T], F32, kind="ExternalOutput")
    hid_o = nc.dram_tensor("hid_o", [S, BL, H], F32, kind="ExternalOutput")
    probs_o = nc.dram_tensor("probs_o", [NCLS, S * BL], F32, kind="ExternalOutput")

    with tile.TileContext(nc) as tc, ExitStack() as ctx:
        dram = ctx.enter_context(tc.tile_pool(name="dram", bufs=1, space="DRAM"))
        # one DRAM tile per (h-chunk, j-block) so every DMA has tiny dep fan-in
        hp_dram = [[dram.tile([128, JBLK], BF16, tag=f"hpd{hc}_{jb}",
                              name=f"hpd{hc}_{jb}") for jb in range(NJ)]
                   for hc in range(4)]

        # ================= PROLOGUE: HpT = W_i2h @ batch_H^T =================
        with (
            tc.tile_pool(name="pro", bufs=1) as pro,
            tc.tile_pool(name="pro_ps", bufs=8, space="PSUM") as pro_ps,
            tc.tile_pool(name="pro_st", bufs=8) as pro_st,
        ):
            wi2h_sb = []
            for ic in range(4):
                w_t = pro.tile([128, H], BF16, tag=f"wi2h{ic}", name=f"w_t{ic}")
                nc.sync.dma_start(w_t[:], wi2h[ic * 128:(ic + 1) * 128, :])
                wi2h_sb.append(w_t)
            bhi_sb = []
            for ic in range(4):
                b_t = pro.tile([128, BT], BF16, tag=f"bhi{ic}", name=f"b_t{ic}")
                nc.sync.dma_start(b_t[:], bh_i[ic * 128:(ic + 1) * 128, :])
                bhi_sb.append(b_t)
            for hc in range(4):
                for ng in range(2):
                    pst = [pro_ps.tile([128, 512], F32, tag="hp", name=f"hp_ps{hc}_{ng}_{i}") for i in range(8)]
                    for ic in range(4):
                        for nb in range(8):
                            n0 = (ng * 8 + nb) * 512
                            nc.tensor.matmul(
                                pst[nb][:],
                                wi2h_sb[ic][:, hc * 128:(hc + 1) * 128],
                                bhi_sb[ic][:, n0:n0 + 512],
                                start=(ic == 0), stop=(ic == 3),
                            )
                    for nb in range(8):
                        nn = ng * 8 + nb
                        st = pro_st.tile([128, 512], BF16, tag="st", name=f"st{hc}_{ng}_{nb}")
                        nc.scalar.copy(st[:], pst[nb][:])
                        nc.gpsimd.dma_start(
                            hp_dram[hc][nn // 2][:, (nn % 2) * 512:(nn % 2 + 1) * 512],
                            st[:])

        # ================= RESIDENT TILES =================
        res = ctx.enter_context(tc.tile_pool(name="res", bufs=1))
        bht_sb = []
        for tcc in range(2):
            t_ = res.tile([128, BL * INP], BF16, tag=f"bht{tcc}", name=f"bht{tcc}")
            nc.sync.dma_start(t_[:], bh_t[tcc * 128:(tcc + 1) * 128, :])
            bht_sb.append(t_)
        wg_sb = []
        for k in range(10):
            t_ = res.tile([128, 4 * H], BF16, tag=f"wg{k}", name=f"wg{k}")
            nc.sync.dma_start(t_[:], wg[k * 128:(k + 1) * 128, :])
            wg_sb.append(t_)
        wh2h_sb = []
        for k in range(4):
            t_ = res.tile([128, H], BF16, tag=f"wh2h{k}", name=f"wh2h{k}")
            nc.sync.dma_start(t_[:], wh2h[k * 128:(k + 1) * 128, :])
            wh2h_sb.append(t_)
        ce_sb = []
        for k in range(2):
            t_ = res.tile([128, S * BL], BF16, tag=f"ce{k}", name=f"ce{k}")
            nc.sync.dma_start(t_[:], ce[k * 128:(k + 1) * 128, :])
            ce_sb.append(t_)
        wgen_sb = []
        for k in range(4):
            t_ = res.tile([128, NCLS], BF16, tag=f"wgen{k}", name=f"wgen{k}")
            nc.sync.dma_start(t_[:], wgen[k * 128:(k + 1) * 128, :])
            wgen_sb.append(t_)
        bgen_sb = res.tile([NCLS, 1], F32, tag="bgen")
        nc.sync.dma_start(bgen_sb[:], bgen[:, :])
        bh2h_sb = res.tile([128, 4], F32, tag="bh2h")
        nc.sync.dma_start(bh2h_sb[:], bh2h[:, :])
        wsc_sb = res.tile([128, 4 * 1024], BF16, tag="wsc")
        nc.sync.dma_start(wsc_sb[:], wsc[:, :])
        bgrow_sb = res.tile([1, 4 * H], BF16, tag="bgrow")
        nc.sync.dma_start(bgrow_sb[:], bgrow[:, :])
        ones_sb = res.tile([1, BL], BF16, tag="ones")
        nc.vector.memset(ones_sb[:], 1.0)
        ident_sb = res.tile([128, 128], BF16, tag="ident")
        masks.make_identity(nc, ident_sb[:])
        hT_all = []
        for k in range(4):
            t_ = res.tile([128, (S + 1) * BL], BF16, tag=f"hT{k}", name=f"hT{k}")
            nc.vector.memset(t_[:, 0:BL], 0.0)
            hT_all.append(t_)
        c_sb = res.tile([BL, H], F32, tag="c")
        nc.vector.memset(c_sb[:], 0.0)
        # phT for step 0 (h=0): just the bias
        phT_sb0 = []
        for hc in range(4):
            t_ = res.tile([128, BL], F32, tag=f"ph0_{hc}", name=f"ph0_{hc}")
            nc.vector.memset(t_[:], 0.0)
            nc.vector.tensor_scalar_add(t_[:], t_[:], bh2h_sb[:, hc:hc + 1])
            phT_sb0.append(t_)

        # ================= LOOP POOLS =================
        hp_pool = ctx.enter_context(tc.tile_pool(name="hp", bufs=2))
        thb_pool = ctx.enter_context(tc.tile_pool(name="thb", bufs=2))
        ps1 = ctx.enter_context(tc.tile_pool(name="ps1", bufs=1, space="PSUM"))
        ps2 = ctx.enter_context(tc.tile_pool(name="ps2", bufs=4, space="PSUM"))
        sbs = ctx.enter_context(tc.tile_pool(name="sbs", bufs=1))
        sb4 = ctx.enter_context(tc.tile_pool(name="sb4", bufs=4))

        phT_cur = phT_sb0

        for s in range(S):
            # ---------- phase A: tanh(HpT + phT) + score ----------
            e_ps = ps1.tile([128, T], F32, tag="mm1")
            for blk in range(NJ):
                j0 = blk * JBLK
                hp_t = [hp_pool.tile([128, JBLK], BF16, tag=f"hp{hc}", name=f"hpt{hc}")
                        for hc in range(4)]
                dma_eng = nc.gpsimd if s == 0 else nc.sync
                for hc in range(4):
                    dma_eng.dma_start(hp_t[hc][:], hp_dram[hc][blk][:])
                th_b = [thb_pool.tile([128, JBLK], BF16, tag=f"thb{hc}", name=f"thb{hc}")
                        for hc in range(4)]
                for hc in range(4):
                    for bb in range(BPB):
                        b = blk * BPB + bb
                        nc.vector.tensor_scalar_add(
                            hp_t[hc][:, bb * T:(bb + 1) * T],
                            hp_t[hc][:, bb * T:(bb + 1) * T],
                            phT_cur[hc][:, b:b + 1])
                    nc.scalar.activation(th_b[hc][:], hp_t[hc][:], AF.Tanh)
                # score for the batch rows of this block (one-hot lhsT
                # places row b of the result; all MMs accumulate into e_ps)
                for bb in range(BPB):
                    b = blk * BPB + bb
                    for hc in range(4):
                        nc.tensor.matmul(
                            e_ps[0:BL, :],
                            wsc_sb[:, hc * 1024 + b * 32:hc * 1024 + (b + 1) * 32],
                            th_b[hc][:, bb * T:(bb + 1) * T],
                            start=(blk == 0 and bb == 0 and hc == 0),
                            stop=(blk == NJ - 1 and bb == BPB - 1 and hc == 3))

            # ---------- softmax over t ----------
            mx = sbs.tile([BL, 1], F32, tag="mx")
            nc.vector.reduce_max(mx[:], e_ps[0:BL, :], axis=AX.X)
            nmx = sbs.tile([BL, 1], F32, tag="nmx")
            nc.vector.tensor_scalar_mul(nmx[:], mx[:], -1.0)
            alpha_bf = sbs.tile([BL, T], BF16, tag="alpha")
            sumexp = sbs.tile([BL, 1], F32, tag="sumexp")
            nc.scalar.activation(alpha_bf[:], e_ps[0:BL, :], AF.Exp,
                                 bias=nmx[:], accum_out=sumexp[:])
            rsum = sbs.tile([BL, 1], F32, tag="rsum")
            nc.vector.reciprocal(rsum[:], sumexp[:])
            # normalized attention map output (off critical path)
            attn_sb = sbs.tile([BL, T], F32, tag="attn")
            nc.vector.tensor_scalar_mul(attn_sb[:], alpha_bf[:], rsum[:])
            nc.gpsimd.dma_start(attn_o[s], attn_sb[:])

            # ---------- alpha^T via PE transpose -> one-hot layout ----------
            aT_oh = []
            for tcc in range(2):
                aT_ps = ps2.tile([128, BL], BF16, tag="tp")
                nc.tensor.transpose(
                    aT_ps[:], alpha_bf[:, tcc * 128:(tcc + 1) * 128],
                    ident_sb[0:BL, 0:BL])
                t_ = sb4.tile([128, BL * 32], BF16, tag="aToh", name=f"aToh{tcc}")
                nc.vector.memset(t_[:], 0.0)
                nc.vector.tensor_copy(t_[:, 0:BL * 32:33], aT_ps[:])
                aT_oh.append(t_)

            # ---------- context = alpha @ batch_H  (unnormalized) ----------
            ctx_ps = ps1.tile([128, INP], F32, tag="mm1")
            for b in range(BL):
                for tcc in range(2):
                    nc.tensor.matmul(
                        ctx_ps[0:BL, :],
                        aT_oh[tcc][:, b * 32:(b + 1) * 32],
                        bht_sb[tcc][:, b * INP:(b + 1) * INP],
                        start=(b == 0 and tcc == 0),
                        stop=(b == BL - 1 and tcc == 1))
            # normalize by 1/sumexp while evacuating, cast bf16
            ctx_bf = sbs.tile([BL, INP], BF16, tag="ctxbf")
            nc.vector.tensor_scalar_mul(ctx_bf[:], ctx_ps[0:BL, :], rsum[:])

            # ---------- ctx^T via PE transpose ----------
            ctxT_sb = []
            for ic in range(4):
                cT_ps = ps2.tile([128, BL], BF16, tag="tp")
                nc.tensor.transpose(
                    cT_ps[:], ctx_bf[:, ic * 128:(ic + 1) * 128],
                    ident_sb[0:BL, 0:BL])
                t_ = sb4.tile([128, BL], BF16, tag="ctxT", name=f"ctxT{ic}")
                nc.vector.tensor_copy(t_[:], cT_ps[:])
                ctxT_sb.append(t_)

            # ---------- gates = [ctx, char_e, h, 1] @ [Wg; bias] ----------
            g_ps = ps1.tile([128, 4 * H], F32, tag="mm1")
            xk = (
                [(ctxT_sb[i][:], 128) for i in range(4)]
                + [(ce_sb[i][:, s * BL:(s + 1) * BL], 128) for i in range(2)]
                + [(hT_all[i][:, s * BL:(s + 1) * BL], 128) for i in range(4)]
            )
            for k, (lhsT, kk) in enumerate(xk):
                for n in range(4):
                    nc.tensor.matmul(
                        g_ps[0:BL, n * 512:(n + 1) * 512],
                        lhsT,
                        wg_sb[k][:, n * 512:(n + 1) * 512],
                        start=(k == 0), stop=False,
                        tile_position=(0, 0))
            for n in range(4):
                nc.tensor.matmul(
                    g_ps[0:BL, n * 512:(n + 1) * 512],
                    ones_sb[:],
                    bgrow_sb[:, n * 512:(n + 1) * 512],
                    start=False, stop=True,
                    tile_position=(0, 0))

            # ---------- gate nonlinearities (tanh only) ----------
            tg = sbs.tile([BL, 4 * H], F32, tag="tg")
            nc.scalar.activation(tg[:, 0:1024], g_ps[0:BL, 0:1024],
                                 AF.Tanh, scale=0.5)
            nc.scalar.activation(tg[:, 1024:1536], g_ps[0:BL, 1024:1536],
                                 AF.Tanh, scale=1.0)
            nc.scalar.activation(tg[:, 1536:2048], g_ps[0:BL, 1536:2048],
                                 AF.Tanh, scale=0.5)

            # ---------- LSTM pointwise ----------
            sig_if = sbs.tile([BL, 1024], F32, tag="sigif")
            nc.vector.tensor_scalar(
                sig_if[:], tg[:, 0:1024], 1.0, 0.5, OP.add, OP.mult)
            m1 = sbs.tile([BL, H], F32, tag="m1")
            nc.vector.tensor_tensor(
                out=m1[:], in0=sig_if[:, 512:1024], in1=c_sb[:], op=OP.mult)
            m2 = sbs.tile([BL, H], F32, tag="m2")
            nc.vector.tensor_tensor(
                out=m2[:], in0=sig_if[:, 0:512], in1=tg[:, 1024:1536],
                op=OP.mult)
            nc.vector.tensor_tensor(
                out=c_sb[:], in0=m1[:], in1=m2[:], op=OP.add)
            tc2_t = sbs.tile([BL, H], F32, tag="tc2")
            nc.scalar.activation(tc2_t[:], c_sb[:], AF.Tanh)
            sig_o = sbs.tile([BL, H], F32, tag="sigo")
            nc.vector.tensor_scalar(
                sig_o[:], tg[:, 1536:2048], 1.0, 0.5, OP.add, OP.mult)
            h2 = sbs.tile([BL, H], F32, tag="h2")
            nc.vector.tensor_tensor(
                out=h2[:], in0=sig_o[:], in1=tc2_t[:], op=OP.mult)
            nc.gpsimd.dma_start(hid_o[s], h2[:])
            h_bf = sbs.tile([BL, H], BF16, tag="hbf")
            nc.vector.tensor_copy(h_bf[:], h2[:])

            # ---------- h^T via PE transpose into hT_all ----------
            for ic in range(4):
                hT_ps = ps2.tile([128, BL], BF16, tag="tp")
                nc.tensor.transpose(
                    hT_ps[:], h_bf[:, ic * 128:(ic + 1) * 128],
                    ident_sb[0:BL, 0:BL])
                nc.vector.tensor_copy(
                    hT_all[ic][:, (s + 1) * BL:(s + 2) * BL], hT_ps[:])

            # ---------- phT for next step ----------
            if s + 1 < S:
                phT_next = []
                for hc in range(4):
                    ph_ps = ps2.tile([128, BL], F32, tag="tp")
                    for kc in range(4):
                        nc.tensor.matmul(
                            ph_ps[:],
                            wh2h_sb[kc][:, hc * 128:(hc + 1) * 128],
                            hT_all[kc][:, (s + 1) * BL:(s + 2) * BL],
                            start=(kc == 0), stop=(kc == 3))
                    t_ = sb4.tile([128, BL], F32, tag="phT", name=f"phT{hc}")
                    nc.scalar.activation(t_[:], ph_ps[:], AF.Identity,
                                         bias=bh2h_sb[:, hc:hc + 1])
                    phT_next.append(t_)
                phT_cur = phT_next

        # ---------- probs = hiddens @ W_gen^T + b_gen ----------
        for sl0, sl1 in ((0, 512), (512, S * BL)):
            pp = ps1.tile([128, sl1 - sl0], F32, tag="mm1")
            for kc in range(4):
                nc.tensor.matmul(
                    pp[0:NCLS, 0:sl1 - sl0],
                    wgen_sb[kc][:],
                    hT_all[kc][:, BL + sl0:BL + sl1],
                    start=(kc == 0), stop=(kc == 3),
                    tile_position=(0, 0))
            pr_sb = sbs.tile([NCLS, 512], F32, tag="prsb")
            nc.scalar.activation(pr_sb[:, 0:sl1 - sl0], pp[0:NCLS, 0:sl1 - sl0],
                                 AF.Identity, bias=bgen_sb[:, 0:1])
            nc.gpsimd.dma_start(probs_o[:, sl0:sl1], pr_sb[:, 0:sl1 - sl0])

    nc.finalize()
    return nc


def _prep_core_inputs(bh, text_np, W_i2h, W_h2h, b_h2h, w_score,
                      W_ih, W_hh, b_ih, b_hh, W_gen, b_gen, emb):
    """Host-side marshalling for one core: layout transposes + bf16 casts."""
    bf = ml_dtypes.bfloat16
    f32 = np.float32
    d = {}
    d["bh_i"] = np.ascontiguousarray(
        bh.transpose(2, 0, 1).reshape(INP, BT)).astype(bf)
    d["bh_t"] = np.ascontiguousarray(
        bh.transpose(1, 0, 2).reshape(T, BL * INP)).astype(bf)
    d["wi2h"] = np.ascontiguousarray(W_i2h.T).astype(bf)          # [IN, H]
    d["wh2h"] = np.ascontiguousarray(W_h2h.T).astype(bf)          # [h', h]
    d["bh2h"] = np.ascontiguousarray(
        b_h2h.reshape(4, 128).T).astype(f32)                      # [128, 4]
    wsc_oh = np.zeros((128, 4, 32, 32), np.float32)
    wch = w_score[0].reshape(4, 128).T                            # [128, 4]
    for b in range(32):
        wsc_oh[:, :, b, b] = wch
    d["wsc"] = wsc_oh.reshape(128, 4096).astype(bf)               # one-hot lhsT
    d["wg"] = np.ascontiguousarray(
        np.concatenate([W_ih, W_hh], axis=1).T).astype(bf)        # [1280, 4H]
    d["bgrow"] = (b_ih + b_hh).reshape(1, 4 * H).astype(bf)
    ce = emb[text_np[:, :S]]                    # [BL, S, E]
    d["ce"] = np.ascontiguousarray(
        ce.transpose(2, 1, 0).reshape(EMB, S * BL)).astype(bf)    # [E, s*BL+b]
    d["wgen"] = np.ascontiguousarray(W_gen.T).astype(bf)          # [H, NCLS]
    d["bgen"] = b_gen.reshape(NCLS, 1).astype(f32)
    return d


def kernel(batch_H, text, batch_max_length, W_i2h, W_h2h, b_h2h, w_score,
           W_ih, W_hh, b_ih, b_hh, W_gen, b_gen, emb):
    from concourse.bass_utils import run_bass_kernel_spmd

    batch_H = np.asarray(batch_H, dtype=np.float32)
    text_np = np.asarray(text)
    assert int(batch_max_length) == S, f"expected {S}, got {batch_max_length}"
    args = [np.asarray(a, dtype=np.float32) for a in
            (W_i2h, W_h2h, b_h2h, w_score, W_ih, W_hh, b_ih, b_hh,
             W_gen, b_gen, emb)]

    if "nc" not in _cache:
        _cache["nc"] = _build_nc()
    nc = _cache["nc"]

    in_maps = []
    for c in range(NCORES):
        bh = batch_H[c * BL:(c + 1) * BL]
        tx = text_np[c * BL:(c + 1) * BL]
        in_maps.append(_prep_core_inputs(bh, tx, *args))

    res = run_bass_kernel_spmd(nc, in_maps, core_ids=list(range(NCORES)))

    probs = np.empty((NCORES * BL, S, NCLS), np.float32)
    attn = np.empty((NCORES * BL, T, S), np.float32)
    hidd = np.empty((NCORES * BL, S, H), np.float32)
    for c in range(NCORES):
        r = res.results[c]
        sl = slice(c * BL, (c + 1) * BL)
        probs[sl] = r["probs_o"].reshape(NCLS, S, BL).transpose(2, 1, 0)
        attn[sl] = r["attn_o"].transpose(1, 2, 0)
        hidd[sl] = r["hid_o"].transpose(1, 0, 2)
    return probs, attn, hidd
